# revision 7
# baseline (speedup 1.0000x reference)
"""Distributed Trainium2 Bass kernel for an attention block.

Reference math (B=2, S=2048, H=2048, NH=16, HD=128):
  qkv = x @ Wqkv.T -> split q,k,v per head -> RoPE(q,k via frequency_cis 2x2)
  scores = (q @ k.T) * 1/sqrt(HD) + mask -> softmax -> @ v -> @ Wout.T

Sharding (8 cores): core c handles batch b=c//4 and heads 4*(c%4)..4*(c%4)+3.
Per core: QKV proj for its 4 heads (bf16), RoPE applied in "rotate-half"
permuted head-dim layout (permutation folded into Wqkv rows on host; softmax
scale folded into Wq rows), attention with numerically stable softmax,
PV computed transposed (outT = v.T-free form) so the attention output lands
as attnT [hd, q]; AllGather over the 4 same-batch cores concatenates the
head dim; out-projection is column-split (each core gets its own 512-column
slice of Wout.T as input), so no rank-dependent indexing exists in the graph.
"""

import numpy as np
import ml_dtypes
from contextlib import ExitStack

B, S, H, NH, HD = 2, 2048, 2048, 16, 128
NHL = 4          # heads per core
NCORES = 8
SCALE = 1.0 / np.sqrt(HD)
BF16 = ml_dtypes.bfloat16

_cache = {}


def _build():
    import concourse.bass as bass
    import concourse.tile as tile
    from concourse import bacc, mybir
    dt = mybir.dt
    nc = bacc.Bacc("TRN2", target_bir_lowering=False, debug=False,
                   num_devices=NCORES)

    xT = nc.dram_tensor("xT", [H, S], dt.bfloat16, kind="ExternalInput").ap()
    wT = nc.dram_tensor("wT", [H, 3 * NHL * HD], dt.bfloat16,
                        kind="ExternalInput").ap()
    rope = nc.dram_tensor("rope", [2, HD, S], dt.float32,
                          kind="ExternalInput").ap()
    mask = nc.dram_tensor("mask", [S, S], dt.float32,
                          kind="ExternalInput").ap()
    ident_in = nc.dram_tensor("ident", [128, 128], dt.bfloat16,
                              kind="ExternalInput").ap()
    attnT_out = nc.dram_tensor("attnT", [NHL * HD, S], dt.bfloat16,
                               kind="ExternalOutput").ap()

    P = 128
    KO = H // P           # 16 contraction chunks
    NQ = S // P           # 16 q blocks
    NK = S // 512         # 4 key 512-tiles

    with tile.TileContext(nc) as tc, ExitStack() as ctx:
        const = ctx.enter_context(tc.tile_pool(name="const", bufs=1))
        ident = const.tile([P, P], dt.bfloat16)
        nc.sync.dma_start(ident[:], ident_in[:])

        # persistent SBUF: roped q/k (bf16), transposed v (bf16)
        qkv_pool = ctx.enter_context(tc.tile_pool(name="qkv", bufs=1))
        qsb = qkv_pool.tile([P, NHL, S], dt.bfloat16, tag="qsb")
        ksb = qkv_pool.tile([P, NHL, S], dt.bfloat16, tag="ksb")
        vsb = qkv_pool.tile([P, NHL, KO, P], dt.bfloat16, tag="vsb")

        # ---------------- Phase 1: QKV projection + RoPE ----------------
        with ExitStack() as p1:
            wpool = p1.enter_context(tc.tile_pool(name="wpool", bufs=1))
            xpool = p1.enter_context(tc.tile_pool(name="xpool", bufs=2))
            rpool = p1.enter_context(tc.tile_pool(name="rpool", bufs=1))
            stg = p1.enter_context(tc.tile_pool(name="stg", bufs=4))
            pmm = p1.enter_context(tc.tile_pool(name="pmm", bufs=2,
                                                space="PSUM"))
            ptr1 = p1.enter_context(tc.tile_pool(name="ptr1", bufs=2,
                                                 space="PSUM"))

            wsb = wpool.tile([P, KO, 3 * NHL * HD], dt.bfloat16)
            nc.sync.dma_start(wsb[:], wT.rearrange("(ko p) m -> p ko m", p=P))
            rsb = rpool.tile([P, 2, S], dt.float32)
            nc.sync.dma_start(rsb[:], rope.rearrange("r p s -> p r s"))

            xTr = xT.rearrange("(ko p) s -> p ko s", p=P)
            for n in range(NK):
                xn = xpool.tile([P, KO, 512], dt.bfloat16, tag="xn")
                nc.sync.dma_start(xn[:], xTr[:, :, n * 512:(n + 1) * 512])
                for h in range(NHL):
                    for t in range(3):   # q, k, v
                        m = (h * 3 + t) * P
                        ps = pmm.tile([P, 512], dt.float32, tag="pmm")
                        for kc in range(KO):
                            nc.tensor.matmul(
                                ps[:], wsb[:, kc, m:m + P], xn[:, kc, :],
                                start=(kc == 0), stop=(kc == KO - 1))
                        ns = slice(n * 512, (n + 1) * 512)
                        if t == 2:       # v: cast + transpose to [s, hd]
                            vt = stg.tile([P, 512], dt.bfloat16, tag="vt")
                            nc.vector.tensor_copy(vt[:], ps[:])
                            for j in range(4):
                                pt = ptr1.tile([P, P], dt.bfloat16, tag="pt")
                                nc.tensor.transpose(
                                    pt[:], vt[:, j * P:(j + 1) * P], ident[:])
                                nc.vector.tensor_copy(
                                    vsb[:, h, n * 4 + j, :], pt[:])
                        else:            # q/k: RoPE in rotate-half layout
                            # rope input holds [A, swap(B)]; u = q*swap(B),
                            # then DMA-swap u's partition halves so
                            # t2 = swap(q)*B, and dst = q*A + t2.
                            dst = qsb if t == 0 else ksb
                            t1 = stg.tile([P, 512], dt.float32, tag="t1")
                            u = stg.tile([P, 512], dt.float32, tag="u")
                            t2 = stg.tile([P, 512], dt.float32, tag="t2")
                            nc.vector.tensor_tensor(
                                t1[:], ps[:], rsb[:, 0, ns],
                                mybir.AluOpType.mult)
                            nc.vector.tensor_tensor(
                                u[:], ps[:], rsb[:, 1, ns],
                                mybir.AluOpType.mult)
                            nc.sync.dma_start(t2[:64], u[64:, :])
                            nc.sync.dma_start(t2[64:], u[:64, :])
                            nc.vector.tensor_tensor(
                                dst[:, h, ns], t1[:], t2[:],
                                mybir.AluOpType.add)

        # ---------------- Phase 2: attention ----------------
        with ExitStack() as p2:
            mpool = p2.enter_context(tc.tile_pool(name="mpool", bufs=2))
            scp = p2.enter_context(tc.tile_pool(name="scp", bufs=2))
            prp = p2.enter_context(tc.tile_pool(name="prp", bufs=2))
            small = p2.enter_context(tc.tile_pool(name="small", bufs=4))
            otp = p2.enter_context(tc.tile_pool(name="otp", bufs=4))
            psc = p2.enter_context(tc.tile_pool(name="psc", bufs=4,
                                                space="PSUM"))
            ptr = p2.enter_context(tc.tile_pool(name="ptr", bufs=2,
                                                space="PSUM"))
            ppv = p2.enter_context(tc.tile_pool(name="ppv", bufs=2,
                                                space="PSUM"))

            for qb in range(NQ):
                mt = mpool.tile([P, S], dt.float32, tag="mt")
                nc.sync.dma_start(mt[:], mask[qb * P:(qb + 1) * P, :])
                qs = slice(qb * P, (qb + 1) * P)
                for h in range(NHL):
                    sc = scp.tile([P, S], dt.float32, tag="sc")
                    for n in range(NK):
                        ns = slice(n * 512, (n + 1) * 512)
                        ps = psc.tile([P, 512], dt.float32, tag="psc")
                        nc.tensor.matmul(ps[:], qsb[:, h, qs], ksb[:, h, ns],
                                         start=True, stop=True)
                        nc.vector.tensor_tensor(sc[:, ns], ps[:], mt[:, ns],
                                                mybir.AluOpType.add)
                    mx = small.tile([P, 1], dt.float32, tag="mx")
                    nc.vector.tensor_reduce(mx[:], sc[:],
                                            axis=mybir.AxisListType.X,
                                            op=mybir.AluOpType.max)
                    nmx = small.tile([P, 1], dt.float32, tag="nmx")
                    nc.vector.tensor_scalar_mul(nmx[:], mx[:], -1.0)
                    pr = prp.tile([P, S], dt.bfloat16, tag="pr")
                    l = small.tile([P, 1], dt.float32, tag="l")
                    nc.scalar.activation(pr[:], sc[:],
                                         mybir.ActivationFunctionType.Exp,
                                         bias=nmx[:], scale=1.0,
                                         accum_out=l[:])
                    rl = small.tile([P, 1], dt.float32, tag="rl")
                    nc.vector.reciprocal(rl[:], l[:])
                    nc.vector.tensor_scalar_mul(pr[:], pr[:], rl[:])
                    # transpose probs 128x128 tiles -> prT [k-part, q]
                    prT = prp.tile([P, KO, P], dt.bfloat16, tag="prT")
                    for kc in range(KO):
                        pt = ptr.tile([P, P], dt.bfloat16, tag="ptt")
                        nc.tensor.transpose(pt[:], pr[:, kc * P:(kc + 1) * P],
                                            ident[:])
                        nc.vector.tensor_copy(prT[:, kc, :], pt[:])
                    # PV: outT[hd, q] += v[s,hd].T-free accumulation
                    po = ppv.tile([P, P], dt.float32, tag="ppv")
                    for kc in range(KO):
                        nc.tensor.matmul(po[:], vsb[:, h, kc, :],
                                         prT[:, kc, :],
                                         start=(kc == 0), stop=(kc == KO - 1))
                    ot = otp.tile([P, P], dt.bfloat16, tag="ot")
                    nc.vector.tensor_copy(ot[:], po[:])
                    nc.sync.dma_start(
                        attnT_out[h * P:(h + 1) * P, qs], ot[:])

    nc.compile()
    return nc


def _build_p2():
    import concourse.bass as bass
    import concourse.tile as tile
    from concourse import bacc, mybir
    dt = mybir.dt
    nc = bacc.Bacc("TRN2", target_bir_lowering=False, debug=False,
                   num_devices=NCORES)
    attnT = nc.dram_tensor("attnT", [H, S], dt.bfloat16,
                           kind="ExternalInput").ap()
    woutT = nc.dram_tensor("woutT", [H, 512], dt.bfloat16,
                           kind="ExternalInput").ap()
    out_ext = nc.dram_tensor("out", [S, 512], dt.float32,
                             kind="ExternalOutput").ap()
    P = 128
    KO = H // P
    NQ = S // P
    with tile.TileContext(nc) as tc, ExitStack() as ctx:
        ap = ctx.enter_context(tc.tile_pool(name="ap", bufs=1))
        wop = ctx.enter_context(tc.tile_pool(name="wop", bufs=1))
        evp = ctx.enter_context(tc.tile_pool(name="evp", bufs=3))
        pmo = ctx.enter_context(tc.tile_pool(name="pmo", bufs=2, space="PSUM"))
        asb = ap.tile([P, KO, S], dt.bfloat16)
        nc.sync.dma_start(asb[:], attnT.rearrange("(ko p) s -> p ko s", p=P))
        wo = wop.tile([P, KO, 512], dt.bfloat16)
        nc.sync.dma_start(wo[:], woutT.rearrange("(ko p) n -> p ko n", p=P))
        for mq in range(NQ):
            po = pmo.tile([P, 512], dt.float32, tag="pmo")
            for kc in range(KO):
                nc.tensor.matmul(po[:], asb[:, kc, mq * P:(mq + 1) * P],
                                 wo[:, kc, :],
                                 start=(kc == 0), stop=(kc == KO - 1))
            ev = evp.tile([P, 512], dt.float32, tag="ev")
            nc.vector.tensor_copy(ev[:], po[:])
            nc.sync.dma_start(out_ext[mq * P:(mq + 1) * P, :], ev[:])
    nc.compile()
    return nc


def _host_prep(x, attention_mask, frequency_cis, Wqkv, Wout):
    """Build the 8 per-core input maps (numpy only)."""
    x = np.asarray(x, dtype=np.float32)
    attention_mask = np.asarray(attention_mask, dtype=np.float32)
    fc = np.asarray(frequency_cis, dtype=np.float32)
    Wqkv = np.asarray(Wqkv, dtype=np.float32)
    Wout = np.asarray(Wout, dtype=np.float32)

    # rotate-half permutation of the head dim: new row p<64 <- old 2p,
    # p>=64 <- old 2(p-64)+1
    perm = np.concatenate([np.arange(0, HD, 2), np.arange(1, HD, 2)])
    # rope coefficients in permuted layout: [A;B] each [HD, S]
    ropeA = np.concatenate([fc[:, :, 0, 0].T, fc[:, :, 1, 1].T], axis=0)
    ropeBsw = np.concatenate([fc[:, :, 1, 0].T, fc[:, :, 0, 1].T], axis=0)
    rope = np.stack([ropeA, ropeBsw]).astype(np.float32)  # [2, HD, S]

    xT = [np.ascontiguousarray(x[b].T).astype(BF16) for b in range(B)]
    woutT_f = Wout.T.astype(np.float32)                  # [H(in), H(out)]

    in_maps = []
    for c in range(NCORES):
        b, g = divmod(c, 4)
        rows = []
        for j in range(NHL):
            hh = (g * NHL + j) * HD
            rows.append(Wqkv[0 * H + hh:0 * H + hh + HD][perm] * SCALE)  # q
            rows.append(Wqkv[1 * H + hh:1 * H + hh + HD][perm])          # k
            rows.append(Wqkv[2 * H + hh:2 * H + hh + HD])                # v
        wloc = np.concatenate(rows, axis=0)              # [1536, H]
        in_maps.append({
            "ident": np.eye(128, dtype=BF16),
            "xT": xT[b],
            "wT": np.ascontiguousarray(wloc.T).astype(BF16),
            "rope": rope,
            "mask": np.ascontiguousarray(attention_mask[b, 0]),
        })
    wout_slices = [np.ascontiguousarray(
        woutT_f[:, g * 512:(g + 1) * 512]).astype(BF16) for g in range(4)]
    return in_maps, wout_slices


def _install_ntff_hook():
    """The image's antenv lacks axon_hooks; shim it so trace=True works."""
    import sys
    import types
    import ctypes
    import contextlib
    if "antenv.axon_hooks" in sys.modules:
        return
    mod = types.ModuleType("antenv.axon_hooks")
    _reg = {"hook": None}
    mod.set_axon_ntff_profile_hook = lambda h: _reg.__setitem__("hook", h)
    mod.get_axon_ntff_profile_hook = lambda: _reg["hook"]
    sys.modules["antenv.axon_hooks"] = mod

    so_path = "/opt/axon/libaxon_pjrt.so"
    try:
        lib = ctypes.CDLL(so_path)
        if not hasattr(lib, "axon_start_nrt_profile"):
            return
        lib.axon_start_nrt_profile.argtypes = [
            ctypes.POINTER(ctypes.c_int64), ctypes.c_size_t]
        lib.axon_start_nrt_profile.restype = ctypes.c_int64
        lib.axon_stop_nrt_profile.argtypes = [ctypes.c_char_p]
        lib.axon_stop_nrt_profile.restype = ctypes.c_int64

        @contextlib.contextmanager
        def _hook(output_dir, device_ids):
            import jax
            jax.devices()
            if device_ids:
                ids = (ctypes.c_int64 * len(device_ids))(*device_ids)
                rc = lib.axon_start_nrt_profile(ids, len(device_ids))
            else:
                rc = lib.axon_start_nrt_profile(None, 0)
            if rc != 0:
                raise RuntimeError(f"axon_start_nrt_profile rc={rc}")
            try:
                yield
            finally:
                n = lib.axon_stop_nrt_profile(str(output_dir).encode())
                print(f"profile: {n} file(s) written to {output_dir}")

        mod.set_axon_ntff_profile_hook(_hook)
    except OSError:
        pass


def _run(in_maps, trace=False):
    if trace:
        _install_ntff_hook()
    from concourse.bass_utils import run_bass_kernel_spmd
    if "nc" not in _cache:
        _cache["nc"] = _build()
        _cache["nc2"] = _build_p2()
    r1 = run_bass_kernel_spmd(_cache["nc"], in_maps[0],
                              list(range(NCORES)), trace=trace)
    attnT_full = [
        np.concatenate([r1.results[4 * b + r]["attnT"] for r in range(4)],
                       axis=0)
        for b in range(B)
    ]
    maps2 = [{"attnT": attnT_full[c // 4], "woutT": in_maps[1][c % 4]}
             for c in range(NCORES)]
    r2 = run_bass_kernel_spmd(_cache["nc2"], maps2,
                              list(range(NCORES)), trace=trace)
    return r1, r2


def kernel(x, attention_mask, frequency_cis, Wqkv, Wout):
    in_maps = _host_prep(x, attention_mask, frequency_cis, Wqkv, Wout)
    _, r2 = _run(in_maps)
    out = np.empty((B, S, H), dtype=np.float32)
    for c in range(NCORES):
        b, g = divmod(c, 4)
        out[b, :, g * 512:(g + 1) * 512] = r2.results[c]["out"]
    return out


def kernel_traced(x, attention_mask, frequency_cis, Wqkv, Wout):
    """Like kernel() but also returns (out, exec_time_ns_total, (t1, t2))."""
    in_maps = _host_prep(x, attention_mask, frequency_cis, Wqkv, Wout)
    r1, r2 = _run(in_maps, trace=True)
    out = np.empty((B, S, H), dtype=np.float32)
    for c in range(NCORES):
        b, g = divmod(c, 4)
        out[b, :, g * 512:(g + 1) * 512] = r2.results[c]["out"]
    t1 = getattr(r1, "exec_time_ns", None)
    t2 = getattr(r2, "exec_time_ns", None)
    tot = (t1 or 0) + (t2 or 0)
    return out, (tot if (t1 or t2) else None), (t1, t2)
